# revision 2
# baseline (speedup 1.0000x reference)
"""MoE layer (8 experts, top-2, D=1024, F=2048) on 8 Trainium2 cores.

Strategy (expert-parallel, per sharding hint):
 - Host: gate matmul + softmax + top-2 (jax on CPU, matching the reference
   op-for-op), then dispatch: gather each expert's tokens, pad to a common
   capacity C, transpose to [D, C].
 - Device (SPMD, core e = expert e): yT = w2_e.T @ (silu(w1_e.T @ xT) *
   (w3_e.T @ xT)). All matmuls keep activations in [feature, token] layout so
   weights load in natural layout and no on-device transposes are needed.
   f32r dtype -> full-rate PE with ~1e-4 matmul precision.
 - Host: combine: out[token] += y * combine_weight (top-2 scatter-add).
"""

import numpy as np
from contextlib import ExitStack

import concourse.bass as bass
import concourse.mybir as mybir
import concourse.tile as tile
from concourse import bacc
from concourse.bass_utils import run_bass_kernel_spmd

E = 8
TOP_K = 2
D = 1024
F = 2048
N_CORES = 8
PART = 128
KD = D // PART  # 8   k-tiles of the D contraction
MF = F // PART  # 16  m-tiles of the F output (mm1/mm3)
KF = F // PART  # 16  k-tiles of the F contraction (mm2)
MD = D // PART  # 8   m-tiles of the D output (mm2)

_F32R = mybir.dt.float32r
_F32 = mybir.dt.float32

TRACE = False  # test harness flips this to profile
_nc_cache: dict[int, object] = {}


def _chunks(C):
    """Split C into psum-width chunks <=512, each >=256 when possible
    (f32r matmul runs at full rate only when the moving free dim >=256)."""
    out, n0, rem = [], 0, C
    while rem > 0:
        if rem <= 512:
            n = rem
        elif rem <= 768:
            n = rem - 256
        else:
            n = 512
        out.append((n0, n))
        n0 += n
        rem -= n
    return out


def _build(C):
    nc = bacc.Bacc(
        trn_type="TRN2", debug=False, enable_asserts=False, num_devices=N_CORES
    )
    xt_d = nc.dram_tensor("xt", [KD, PART, C], _F32R, kind="ExternalInput")
    w1_d = nc.dram_tensor("w1t", [MF, PART, KD * PART], _F32R, kind="ExternalInput")
    w3_d = nc.dram_tensor("w3t", [MF, PART, KD * PART], _F32R, kind="ExternalInput")
    w2_d = nc.dram_tensor("w2t", [MD, PART, KF * PART], _F32R, kind="ExternalInput")
    yt_d = nc.dram_tensor("yt", [MD, PART, C], _F32, kind="ExternalOutput")
    chunks = _chunks(C)

    with tile.TileContext(nc) as tc, ExitStack() as ctx:
        xpool = ctx.enter_context(tc.tile_pool(name="x", bufs=KD))
        hpool = ctx.enter_context(tc.tile_pool(name="h", bufs=MF))
        w1pool = ctx.enter_context(tc.tile_pool(name="w1", bufs=2))
        w3pool = ctx.enter_context(tc.tile_pool(name="w3", bufs=2))
        w2pool = ctx.enter_context(tc.tile_pool(name="w2", bufs=2))
        tmppool = ctx.enter_context(tc.tile_pool(name="tmp", bufs=3))
        ypool = ctx.enter_context(tc.tile_pool(name="y", bufs=2))
        p1pool = ctx.enter_context(tc.tile_pool(name="p1", bufs=2, space="PSUM"))
        p3pool = ctx.enter_context(tc.tile_pool(name="p3", bufs=2, space="PSUM"))
        p2pool = ctx.enter_context(tc.tile_pool(name="p2", bufs=2, space="PSUM"))

        # Resident xT k-tiles [128, C] and hT tiles [128, C]
        xts = []
        for kd in range(KD):
            t = xpool.tile([PART, C], _F32R, tag="x", name=f"xt{kd}")
            nc.sync.dma_start(t[:], xt_d.ap()[kd])
            xts.append(t)
        hts = [hpool.tile([PART, C], _F32R, tag="h", name=f"ht{i}") for i in range(MF)]

        # Phase 1: hT[mf] = silu(w1.T @ xT) * (w3.T @ xT), weight-stationary
        for mf in range(MF):
            w1t = w1pool.tile([PART, KD * PART], _F32R, tag="w1")
            nc.sync.dma_start(w1t[:], w1_d.ap()[mf])
            w3t = w3pool.tile([PART, KD * PART], _F32R, tag="w3")
            nc.sync.dma_start(w3t[:], w3_d.ap()[mf])
            for n0, n in chunks:
                p1 = p1pool.tile([PART, 512], _F32, tag="p1")
                p3 = p3pool.tile([PART, 512], _F32, tag="p3")
                for kd in range(KD):
                    nc.tensor.matmul(
                        p1[:, :n],
                        w1t[:, bass.ts(kd, PART)],
                        xts[kd][:, n0 : n0 + n],
                        start=(kd == 0),
                        stop=(kd == KD - 1),
                    )
                for kd in range(KD):
                    nc.tensor.matmul(
                        p3[:, :n],
                        w3t[:, bass.ts(kd, PART)],
                        xts[kd][:, n0 : n0 + n],
                        start=(kd == 0),
                        stop=(kd == KD - 1),
                    )
                tmp = tmppool.tile([PART, 512], _F32R, tag="tmp")
                nc.scalar.activation(
                    tmp[:, :n], p1[:, :n], mybir.ActivationFunctionType.Silu
                )
                nc.vector.tensor_mul(hts[mf][:, n0 : n0 + n], tmp[:, :n], p3[:, :n])

        # Phase 2: yT[md] = w2.T @ hT
        for md in range(MD):
            w2t = w2pool.tile([PART, KF * PART], _F32R, tag="w2")
            nc.sync.dma_start(w2t[:], w2_d.ap()[md])
            yt = ypool.tile([PART, C], _F32, tag="y")
            for n0, n in chunks:
                p2 = p2pool.tile([PART, 512], _F32, tag="p2")
                for kf in range(KF):
                    nc.tensor.matmul(
                        p2[:, :n],
                        w2t[:, bass.ts(kf, PART)],
                        hts[kf][:, n0 : n0 + n],
                        start=(kf == 0),
                        stop=(kf == KF - 1),
                    )
                nc.vector.tensor_copy(yt[:, n0 : n0 + n], p2[:, :n])
            nc.sync.dma_start(yt_d.ap()[md], yt[:])

    nc.compile()
    return nc


def _get_nc(C):
    if C not in _nc_cache:
        _nc_cache[C] = _build(C)
    return _nc_cache[C]


def _route(x, gate_w):
    """Gate + top-2 routing, matching reference numerics (jax on CPU)."""
    import jax
    import jax.numpy as jnp

    cpu = jax.devices("cpu")[0]
    with jax.default_device(cpu):
        xj = jnp.asarray(x)
        gj = jnp.asarray(gate_w)
        probs = jax.nn.softmax(xj @ gj, axis=-1)
        top_w, top_i = jax.lax.top_k(probs, TOP_K)
        top_w = top_w / jnp.sum(top_w, axis=-1, keepdims=True)
        return np.asarray(top_w), np.asarray(top_i)


def _pack_w(w, m_tiles, k_tiles):
    """[K, M] -> [m_tiles, 128, k_tiles*128]: per m-tile, SBUF layout
    [k-partition(128), (k-tile, m-col)]."""
    return np.ascontiguousarray(
        w.reshape(k_tiles, PART, m_tiles, PART)
        .transpose(2, 1, 0, 3)
        .reshape(m_tiles, PART, k_tiles * PART)
    )


def kernel(hidden_states, gate_w, w1, w2, w3):
    hidden_states = np.asarray(hidden_states, dtype=np.float32)
    gate_w = np.asarray(gate_w, dtype=np.float32)
    w1 = np.asarray(w1, dtype=np.float32)
    w2 = np.asarray(w2, dtype=np.float32)
    w3 = np.asarray(w3, dtype=np.float32)

    orig_shape = hidden_states.shape
    x = hidden_states.reshape(-1, D)  # [T, D]
    T = x.shape[0]

    top_w, top_i = _route(x, gate_w)

    # Dispatch: group (token, slot) pairs by expert, token order preserved
    flat_e = top_i.reshape(-1)
    flat_w = top_w.reshape(-1)
    tok_of = np.repeat(np.arange(T), TOP_K)
    order = np.argsort(flat_e, kind="stable")
    sorted_tok = tok_of[order]
    sorted_w = flat_w[order]
    counts = np.bincount(flat_e, minlength=E)
    starts = np.concatenate([[0], np.cumsum(counts)])
    C = max(256, int(np.ceil(counts.max() / PART)) * PART)

    in_maps = []
    for e in range(E):
        idx = sorted_tok[starts[e] : starts[e + 1]]
        xt = np.zeros((D, C), np.float32)
        xt[:, : len(idx)] = x[idx].T
        in_maps.append(
            {
                "xt": np.ascontiguousarray(xt.reshape(KD, PART, C)),
                "w1t": _pack_w(w1[e], MF, KD),
                "w3t": _pack_w(w3[e], MF, KD),
                "w2t": _pack_w(w2[e], MD, KF),
            }
        )

    nc = _get_nc(C)
    res = run_bass_kernel_spmd(
        nc, in_maps, core_ids=list(range(N_CORES)), trace=TRACE
    )
    kernel.last_result = res

    out = np.zeros((T, D), np.float32)
    for e in range(E):
        idx = sorted_tok[starts[e] : starts[e + 1]]
        wts = sorted_w[starts[e] : starts[e + 1]]
        yt = res.results[e]["yt"].reshape(D, C)
        out[idx] += yt[:, : len(idx)].T * wts[:, None]
    return out.reshape(orig_shape)


# revision 4
# speedup vs baseline: 1.0095x; 1.0095x over previous
"""MoE layer (8 experts, top-2, D=1024, F=2048) on 8 Trainium2 cores.

Strategy (expert-parallel, per sharding hint):
 - Host: gate matmul + softmax + top-2 (jax on CPU, matching the reference
   op-for-op), then dispatch: gather each expert's tokens, pad to a common
   capacity C, transpose to [D, C].
 - Device (SPMD, core e = expert e): yT = w2_e.T @ (silu(w1_e.T @ xT) *
   (w3_e.T @ xT)). All matmuls keep activations in [feature, token] layout so
   weights load in natural layout and no on-device transposes are needed.
   f32r dtype -> full-rate PE with ~1e-4 matmul precision.
 - Host: combine: out[token] += y * combine_weight (top-2 scatter-add).
"""

import numpy as np
from contextlib import ExitStack

import concourse.bass as bass
import concourse.mybir as mybir
import concourse.tile as tile
from concourse import bacc
from concourse.bass_utils import run_bass_kernel_spmd

E = 8
TOP_K = 2
D = 1024
F = 2048
N_CORES = 8
PART = 128
KD = D // PART  # 8   k-tiles of the D contraction
MF = F // PART  # 16  m-tiles of the F output (mm1/mm3)
KF = F // PART  # 16  k-tiles of the F contraction (mm2)
MD = D // PART  # 8   m-tiles of the D output (mm2)

_F32R = mybir.dt.float32r
_F32 = mybir.dt.float32

TRACE = False  # test harness flips this to profile
_nc_cache: dict[int, object] = {}


def _chunks(C):
    """Split C into near-equal psum-width chunks <=512. Equal sizes keep every
    matmul's streaming time >= its LDWEIGHTS time (f32r has no fast weight
    load), so weight loads stay hidden; each chunk must be >=256 for f32r
    full-rate."""
    nch = max(1, -(-C // 512))
    base = C // nch
    sizes = [base + (1 if i < C - base * nch else 0) for i in range(nch)]
    out, n0 = [], 0
    for n in sizes:
        out.append((n0, n))
        n0 += n
    return out


def _build(C):
    nc = bacc.Bacc(
        trn_type="TRN2", debug=False, enable_asserts=False, num_devices=N_CORES
    )
    xt_d = nc.dram_tensor("xt", [KD, PART, C], _F32R, kind="ExternalInput")
    w1_d = nc.dram_tensor("w1t", [MF, PART, KD * PART], _F32R, kind="ExternalInput")
    w3_d = nc.dram_tensor("w3t", [MF, PART, KD * PART], _F32R, kind="ExternalInput")
    w2_d = nc.dram_tensor("w2t", [MD, PART, KF * PART], _F32R, kind="ExternalInput")
    yt_d = nc.dram_tensor("yt", [MD, PART, C], _F32, kind="ExternalOutput")
    chunks = _chunks(C)

    with tile.TileContext(nc) as tc, ExitStack() as ctx:
        xpool = ctx.enter_context(tc.tile_pool(name="x", bufs=KD))
        hpool = ctx.enter_context(tc.tile_pool(name="h", bufs=MF))
        w1pool = ctx.enter_context(tc.tile_pool(name="w1", bufs=2))
        w3pool = ctx.enter_context(tc.tile_pool(name="w3", bufs=2))
        w2pool = ctx.enter_context(tc.tile_pool(name="w2", bufs=2))
        tmppool = ctx.enter_context(tc.tile_pool(name="tmp", bufs=3))
        ypool = ctx.enter_context(tc.tile_pool(name="y", bufs=2))
        p1pool = ctx.enter_context(tc.tile_pool(name="p1", bufs=3, space="PSUM"))
        p3pool = ctx.enter_context(tc.tile_pool(name="p3", bufs=3, space="PSUM"))
        p2pool = ctx.enter_context(tc.tile_pool(name="p2", bufs=2, space="PSUM"))

        # Resident xT k-tiles [128, C] and hT tiles [128, C]
        xts = []
        for kd in range(KD):
            t = xpool.tile([PART, C], _F32R, tag="x", name=f"xt{kd}")
            nc.sync.dma_start(t[:], xt_d.ap()[kd])
            xts.append(t)
        hts = [hpool.tile([PART, C], _F32R, tag="h", name=f"ht{i}") for i in range(MF)]

        # Phase 1: hT[mf] = silu(w1.T @ xT) * (w3.T @ xT), weight-stationary
        for mf in range(MF):
            w1t = w1pool.tile([PART, KD * PART], _F32R, tag="w1")
            nc.sync.dma_start(w1t[:], w1_d.ap()[mf])
            w3t = w3pool.tile([PART, KD * PART], _F32R, tag="w3")
            nc.sync.dma_start(w3t[:], w3_d.ap()[mf])
            for n0, n in chunks:
                p1 = p1pool.tile([PART, 512], _F32, tag="p1")
                p3 = p3pool.tile([PART, 512], _F32, tag="p3")
                for kd in range(KD):
                    nc.tensor.matmul(
                        p1[:, :n],
                        w1t[:, bass.ts(kd, PART)],
                        xts[kd][:, n0 : n0 + n],
                        start=(kd == 0),
                        stop=(kd == KD - 1),
                    )
                for kd in range(KD):
                    nc.tensor.matmul(
                        p3[:, :n],
                        w3t[:, bass.ts(kd, PART)],
                        xts[kd][:, n0 : n0 + n],
                        start=(kd == 0),
                        stop=(kd == KD - 1),
                    )
                tmp = tmppool.tile([PART, 512], _F32R, tag="tmp")
                nc.scalar.activation(
                    tmp[:, :n], p1[:, :n], mybir.ActivationFunctionType.Silu
                )
                nc.vector.tensor_mul(hts[mf][:, n0 : n0 + n], tmp[:, :n], p3[:, :n])

        # Phase 2: yT[md] = w2.T @ hT
        for md in range(MD):
            w2t = w2pool.tile([PART, KF * PART], _F32R, tag="w2")
            nc.sync.dma_start(w2t[:], w2_d.ap()[md])
            yt = ypool.tile([PART, C], _F32, tag="y")
            for n0, n in chunks:
                p2 = p2pool.tile([PART, 512], _F32, tag="p2")
                for kf in range(KF):
                    nc.tensor.matmul(
                        p2[:, :n],
                        w2t[:, bass.ts(kf, PART)],
                        hts[kf][:, n0 : n0 + n],
                        start=(kf == 0),
                        stop=(kf == KF - 1),
                    )
                nc.vector.tensor_copy(yt[:, n0 : n0 + n], p2[:, :n])
            nc.sync.dma_start(yt_d.ap()[md], yt[:])

    nc.compile()
    return nc


def _get_nc(C):
    if C not in _nc_cache:
        _nc_cache[C] = _build(C)
    return _nc_cache[C]


def _route(x, gate_w):
    """Gate + top-2 routing, matching reference numerics (jax on CPU)."""
    import jax
    import jax.numpy as jnp

    cpu = jax.devices("cpu")[0]
    with jax.default_device(cpu):
        xj = jnp.asarray(x)
        gj = jnp.asarray(gate_w)
        probs = jax.nn.softmax(xj @ gj, axis=-1)
        top_w, top_i = jax.lax.top_k(probs, TOP_K)
        top_w = top_w / jnp.sum(top_w, axis=-1, keepdims=True)
        return np.asarray(top_w), np.asarray(top_i)


def _pack_w(w, m_tiles, k_tiles):
    """[K, M] -> [m_tiles, 128, k_tiles*128]: per m-tile, SBUF layout
    [k-partition(128), (k-tile, m-col)]."""
    return np.ascontiguousarray(
        w.reshape(k_tiles, PART, m_tiles, PART)
        .transpose(2, 1, 0, 3)
        .reshape(m_tiles, PART, k_tiles * PART)
    )


def kernel(hidden_states, gate_w, w1, w2, w3):
    hidden_states = np.asarray(hidden_states, dtype=np.float32)
    gate_w = np.asarray(gate_w, dtype=np.float32)
    w1 = np.asarray(w1, dtype=np.float32)
    w2 = np.asarray(w2, dtype=np.float32)
    w3 = np.asarray(w3, dtype=np.float32)

    orig_shape = hidden_states.shape
    x = hidden_states.reshape(-1, D)  # [T, D]
    T = x.shape[0]

    top_w, top_i = _route(x, gate_w)

    # Dispatch: group (token, slot) pairs by expert, token order preserved
    flat_e = top_i.reshape(-1)
    flat_w = top_w.reshape(-1)
    tok_of = np.repeat(np.arange(T), TOP_K)
    order = np.argsort(flat_e, kind="stable")
    sorted_tok = tok_of[order]
    sorted_w = flat_w[order]
    counts = np.bincount(flat_e, minlength=E)
    starts = np.concatenate([[0], np.cumsum(counts)])
    C = max(256, int(np.ceil(counts.max() / PART)) * PART)

    in_maps = []
    for e in range(E):
        idx = sorted_tok[starts[e] : starts[e + 1]]
        xt = np.zeros((D, C), np.float32)
        xt[:, : len(idx)] = x[idx].T
        in_maps.append(
            {
                "xt": np.ascontiguousarray(xt.reshape(KD, PART, C)),
                "w1t": _pack_w(w1[e], MF, KD),
                "w3t": _pack_w(w3[e], MF, KD),
                "w2t": _pack_w(w2[e], MD, KF),
            }
        )

    nc = _get_nc(C)
    res = run_bass_kernel_spmd(
        nc, in_maps, core_ids=list(range(N_CORES)), trace=TRACE
    )
    kernel.last_result = res

    out = np.zeros((T, D), np.float32)
    for e in range(E):
        idx = sorted_tok[starts[e] : starts[e + 1]]
        wts = sorted_w[starts[e] : starts[e + 1]]
        yt = res.results[e]["yt"].reshape(D, C)
        out[idx] += yt[:, : len(idx)].T * wts[:, None]
    return out.reshape(orig_shape)


# revision 9
# speedup vs baseline: 1.0633x; 1.0533x over previous
"""MoE layer (8 experts, top-2, D=1024, F=2048) on 8 Trainium2 cores.

Strategy (expert-parallel, per sharding hint):
 - Host: gate matmul + softmax + top-2 (jax on CPU, matching the reference
   op-for-op), then dispatch: gather each expert's tokens, pad to a common
   capacity C, transpose to [D, C].
 - Device (SPMD, core e = expert e): yT = w2_e.T @ (silu(w1_e.T @ xT) *
   (w3_e.T @ xT)). All matmuls keep activations in [feature, token] layout so
   weights load in natural layout and no on-device transposes are needed.
   f32r dtype -> full-rate PE with ~1e-4 matmul precision.
 - Host: combine: out[token] += y * combine_weight (top-2 scatter-add).
"""

import numpy as np
from contextlib import ExitStack

import concourse.bass as bass
import concourse.mybir as mybir
import concourse.tile as tile
from concourse import bacc
from concourse.bass_utils import run_bass_kernel_spmd

E = 8
TOP_K = 2
D = 1024
F = 2048
N_CORES = 8
PART = 128
KD = D // PART  # 8   k-tiles of the D contraction
MF = F // PART  # 16  m-tiles of the F output (mm1/mm3)
KF = F // PART  # 16  k-tiles of the F contraction (mm2)
MD = D // PART  # 8   m-tiles of the D output (mm2)

_F32R = mybir.dt.float32r
_F32 = mybir.dt.float32

TRACE = False  # test harness flips this to profile
_nc_cache: dict[int, object] = {}


def _chunks(C):
    """Split C into near-equal psum-width chunks <=512. Equal sizes keep every
    matmul's streaming time >= its LDWEIGHTS time (f32r has no fast weight
    load), so weight loads stay hidden; each chunk must be >=256 for f32r
    full-rate."""
    assert C % 2 == 0  # f32r ISA: moving-dim element counts must be even
    u = C // 2
    nch = max(1, -(-C // 512))
    base = u // nch
    sizes = [2 * (base + (1 if i < u - base * nch else 0)) for i in range(nch)]
    out, n0 = [], 0
    for n in sizes:
        out.append((n0, n))
        n0 += n
    return out


def _build(C):
    nc = bacc.Bacc(
        trn_type="TRN2", debug=False, enable_asserts=False, num_devices=N_CORES
    )
    xt_d = nc.dram_tensor("xt", [KD, PART, C], _F32R, kind="ExternalInput")
    w1_d = nc.dram_tensor("w1t", [MF, PART, KD * PART], _F32R, kind="ExternalInput")
    w3_d = nc.dram_tensor("w3t", [MF, PART, KD * PART], _F32R, kind="ExternalInput")
    w2_d = nc.dram_tensor("w2t", [MD, PART, KF * PART], _F32R, kind="ExternalInput")
    yt_d = nc.dram_tensor("yt", [MD, PART, C], _F32, kind="ExternalOutput")
    chunks = _chunks(C)

    with tile.TileContext(nc) as tc, ExitStack() as ctx:
        xpool = ctx.enter_context(tc.tile_pool(name="x", bufs=KD))
        hpool = ctx.enter_context(tc.tile_pool(name="h", bufs=MF))
        w1pool = ctx.enter_context(tc.tile_pool(name="w1", bufs=2))
        w3pool = ctx.enter_context(tc.tile_pool(name="w3", bufs=2))
        w2pool = ctx.enter_context(tc.tile_pool(name="w2", bufs=2))
        tmppool = ctx.enter_context(tc.tile_pool(name="tmp", bufs=3))
        ypool = ctx.enter_context(tc.tile_pool(name="y", bufs=2))
        p1pool = ctx.enter_context(tc.tile_pool(name="p1", bufs=3, space="PSUM"))
        p3pool = ctx.enter_context(tc.tile_pool(name="p3", bufs=3, space="PSUM"))
        p2pool = ctx.enter_context(tc.tile_pool(name="p2", bufs=2, space="PSUM"))

        # First weight tiles up front: these gate the very first matmuls, so
        # their descriptors must land at the head of the DMA queues.
        w1t0 = w1pool.tile([PART, KD * PART], _F32R, tag="w1", name="w1t0")
        nc.sync.dma_start(w1t0[:], w1_d.ap()[0])
        w3t0 = w3pool.tile([PART, KD * PART], _F32R, tag="w3", name="w3t0")
        nc.sync.dma_start(w3t0[:], w3_d.ap()[0])

        # Resident xT k-tiles [128, C], loaded chunk-major so the first
        # chunk's columns (what the first matmul group reads) arrive first.
        xts = [xpool.tile([PART, C], _F32R, tag="x", name=f"xt{kd}") for kd in range(KD)]
        for n0, n in chunks:
            for kd in range(KD):
                nc.sync.dma_start(
                    xts[kd][:, n0 : n0 + n], xt_d.ap()[kd][:, n0 : n0 + n]
                )
        hts = [hpool.tile([PART, C], _F32R, tag="h", name=f"ht{i}") for i in range(MF)]

        # Phase 1: hT[mf] = silu(w1.T @ xT) * (w3.T @ xT), weight-stationary
        for mf in range(MF):
            if mf == 0:
                w1t, w3t = w1t0, w3t0
            else:
                w1t = w1pool.tile([PART, KD * PART], _F32R, tag="w1")
                nc.sync.dma_start(w1t[:], w1_d.ap()[mf])
                w3t = w3pool.tile([PART, KD * PART], _F32R, tag="w3")
                nc.sync.dma_start(w3t[:], w3_d.ap()[mf])
            for n0, n in chunks:
                p1 = p1pool.tile([PART, 512], _F32, tag="p1")
                p3 = p3pool.tile([PART, 512], _F32, tag="p3")
                for kd in range(KD):
                    nc.tensor.matmul(
                        p1[:, :n],
                        w1t[:, bass.ts(kd, PART)],
                        xts[kd][:, n0 : n0 + n],
                        start=(kd == 0),
                        stop=(kd == KD - 1),
                    )
                for kd in range(KD):
                    nc.tensor.matmul(
                        p3[:, :n],
                        w3t[:, bass.ts(kd, PART)],
                        xts[kd][:, n0 : n0 + n],
                        start=(kd == 0),
                        stop=(kd == KD - 1),
                    )
                tmp = tmppool.tile([PART, 512], _F32R, tag="tmp")
                nc.scalar.activation(
                    tmp[:, :n], p1[:, :n], mybir.ActivationFunctionType.Silu
                )
                nc.vector.tensor_mul(hts[mf][:, n0 : n0 + n], tmp[:, :n], p3[:, :n])

        # Phase 2: yT[md] = w2.T @ hT
        for md in range(MD):
            w2t = w2pool.tile([PART, KF * PART], _F32R, tag="w2")
            nc.sync.dma_start(w2t[:], w2_d.ap()[md])
            yt = ypool.tile([PART, C], _F32, tag="y")
            for n0, n in chunks:
                p2 = p2pool.tile([PART, 512], _F32, tag="p2")
                for kf in range(KF):
                    nc.tensor.matmul(
                        p2[:, :n],
                        w2t[:, bass.ts(kf, PART)],
                        hts[kf][:, n0 : n0 + n],
                        start=(kf == 0),
                        stop=(kf == KF - 1),
                    )
                nc.vector.tensor_copy(yt[:, n0 : n0 + n], p2[:, :n])
                nc.sync.dma_start(
                    yt_d.ap()[md][:, n0 : n0 + n], yt[:, n0 : n0 + n]
                )

    nc.compile()
    return nc


def _get_nc(C):
    if C not in _nc_cache:
        _nc_cache[C] = _build(C)
    return _nc_cache[C]


def _route(x, gate_w):
    """Gate + top-2 routing, matching reference numerics (jax on CPU)."""
    import jax
    import jax.numpy as jnp

    cpu = jax.devices("cpu")[0]
    with jax.default_device(cpu):
        xj = jnp.asarray(x)
        gj = jnp.asarray(gate_w)
        probs = jax.nn.softmax(xj @ gj, axis=-1)
        top_w, top_i = jax.lax.top_k(probs, TOP_K)
        top_w = top_w / jnp.sum(top_w, axis=-1, keepdims=True)
        return np.asarray(top_w), np.asarray(top_i)


def _pack_w(w, m_tiles, k_tiles):
    """[K, M] -> [m_tiles, 128, k_tiles*128]: per m-tile, SBUF layout
    [k-partition(128), (k-tile, m-col)]."""
    return np.ascontiguousarray(
        w.reshape(k_tiles, PART, m_tiles, PART)
        .transpose(2, 1, 0, 3)
        .reshape(m_tiles, PART, k_tiles * PART)
    )


def kernel(hidden_states, gate_w, w1, w2, w3):
    hidden_states = np.asarray(hidden_states, dtype=np.float32)
    gate_w = np.asarray(gate_w, dtype=np.float32)
    w1 = np.asarray(w1, dtype=np.float32)
    w2 = np.asarray(w2, dtype=np.float32)
    w3 = np.asarray(w3, dtype=np.float32)

    orig_shape = hidden_states.shape
    x = hidden_states.reshape(-1, D)  # [T, D]
    T = x.shape[0]

    top_w, top_i = _route(x, gate_w)

    # Dispatch: group (token, slot) pairs by expert, token order preserved
    flat_e = top_i.reshape(-1)
    flat_w = top_w.reshape(-1)
    tok_of = np.repeat(np.arange(T), TOP_K)
    order = np.argsort(flat_e, kind="stable")
    sorted_tok = tok_of[order]
    sorted_w = flat_w[order]
    counts = np.bincount(flat_e, minlength=E)
    starts = np.concatenate([[0], np.cumsum(counts)])
    C = max(256, int(counts.max() + 1) // 2 * 2)  # round to even (f32r ISA)

    in_maps = []
    for e in range(E):
        idx = sorted_tok[starts[e] : starts[e + 1]]
        xt = np.zeros((D, C), np.float32)
        xt[:, : len(idx)] = x[idx].T
        in_maps.append(
            {
                "xt": np.ascontiguousarray(xt.reshape(KD, PART, C)),
                "w1t": _pack_w(w1[e], MF, KD),
                "w3t": _pack_w(w3[e], MF, KD),
                "w2t": _pack_w(w2[e], MD, KF),
            }
        )

    nc = _get_nc(C)
    res = run_bass_kernel_spmd(
        nc, in_maps, core_ids=list(range(N_CORES)), trace=TRACE
    )
    kernel.last_result = res

    out = np.zeros((T, D), np.float32)
    for e in range(E):
        idx = sorted_tok[starts[e] : starts[e + 1]]
        wts = sorted_w[starts[e] : starts[e + 1]]
        yt = res.results[e]["yt"].reshape(D, C)
        out[idx] += yt[:, : len(idx)].T * wts[:, None]
    return out.reshape(orig_shape)


# revision 12
# speedup vs baseline: 1.1877x; 1.1169x over previous
"""MoE layer (8 experts, top-2, D=1024, F=2048) on 8 Trainium2 cores.

Strategy (expert-parallel, per sharding hint):
 - Host: gate matmul + softmax + top-2 (jax on CPU, matching the reference
   op-for-op), then dispatch: gather each expert's tokens, pad to a common
   capacity C, transpose to [D, C].
 - Device (SPMD, core e = expert e): yT = w2_e.T @ (silu(w1_e.T @ xT) *
   (w3_e.T @ xT)). All matmuls keep activations in [feature, token] layout so
   weights load in natural layout and no on-device transposes are needed.
   f32r dtype -> full-rate PE with ~1e-4 matmul precision.
 - Host: combine: out[token] += y * combine_weight (top-2 scatter-add).
"""

import numpy as np
from contextlib import ExitStack

import concourse.bass as bass
import concourse.mybir as mybir
import concourse.tile as tile
from concourse import bacc
from concourse.bass_utils import run_bass_kernel_spmd

E = 8
TOP_K = 2
D = 1024
F = 2048
N_CORES = 8
PART = 128
KD = D // PART  # 8   k-tiles of the D contraction
MF = F // PART  # 16  m-tiles of the F output (mm1/mm3)
KF = F // PART  # 16  k-tiles of the F contraction (mm2)
MD = D // PART  # 8   m-tiles of the D output (mm2)

_F32R = mybir.dt.float32r
_F32 = mybir.dt.float32

TRACE = False  # test harness flips this to profile
_nc_cache: dict[int, object] = {}


def _chunks(C):
    """Split C into near-equal psum-width chunks <=512. Equal sizes keep every
    matmul's streaming time >= its LDWEIGHTS time (f32r has no fast weight
    load), so weight loads stay hidden; each chunk must be >=256 for f32r
    full-rate."""
    assert C % 2 == 0  # f32r ISA: moving-dim element counts must be even
    u = C // 2
    nch = max(1, -(-C // 512))
    base = u // nch
    sizes = [2 * (base + (1 if i < u - base * nch else 0)) for i in range(nch)]
    out, n0 = [], 0
    for n in sizes:
        out.append((n0, n))
        n0 += n
    return out


def _build(C):
    nc = bacc.Bacc(
        trn_type="TRN2", debug=False, enable_asserts=False, num_devices=N_CORES
    )
    xt_d = nc.dram_tensor("xt", [KD, PART, C], _F32R, kind="ExternalInput")
    w1_d = nc.dram_tensor("w1t", [MF, PART, KD * PART], _F32R, kind="ExternalInput")
    w3_d = nc.dram_tensor("w3t", [MF, PART, KD * PART], _F32R, kind="ExternalInput")
    w2_d = nc.dram_tensor("w2t", [MD, PART, KF * PART], _F32R, kind="ExternalInput")
    yt_d = nc.dram_tensor("yt", [MD, PART, C], _F32, kind="ExternalOutput")
    chunks = _chunks(C)

    with tile.TileContext(nc) as tc, ExitStack() as ctx:
        xpool = ctx.enter_context(tc.tile_pool(name="x", bufs=KD))
        hpool = ctx.enter_context(tc.tile_pool(name="h", bufs=MF))
        w1pool = ctx.enter_context(tc.tile_pool(name="w1", bufs=2))
        w3pool = ctx.enter_context(tc.tile_pool(name="w3", bufs=2))
        w2pool = ctx.enter_context(tc.tile_pool(name="w2", bufs=2))
        tmppool = ctx.enter_context(tc.tile_pool(name="tmp", bufs=3))
        ypool = ctx.enter_context(tc.tile_pool(name="y", bufs=2))
        p1pool = ctx.enter_context(tc.tile_pool(name="p1", bufs=3, space="PSUM"))
        p3pool = ctx.enter_context(tc.tile_pool(name="p3", bufs=3, space="PSUM"))
        p2pool = ctx.enter_context(tc.tile_pool(name="p2", bufs=2, space="PSUM"))

        # First weight tiles up front: these gate the very first matmuls, so
        # their descriptors must land at the head of the DMA queues.
        w1t0 = w1pool.tile([PART, KD * PART], _F32R, tag="w1", name="w1t0")
        nc.sync.dma_start(w1t0[:], w1_d.ap()[0])
        w3t0 = w3pool.tile([PART, KD * PART], _F32R, tag="w3", name="w3t0")
        nc.sync.dma_start(w3t0[:], w3_d.ap()[0])

        # Resident xT k-tiles [128, C], loaded chunk-major so the first
        # chunk's columns (what the first matmul group reads) arrive first.
        xts = [xpool.tile([PART, C], _F32R, tag="x", name=f"xt{kd}") for kd in range(KD)]
        for n0, n in chunks:
            for kd in range(KD):
                nc.sync.dma_start(
                    xts[kd][:, n0 : n0 + n], xt_d.ap()[kd][:, n0 : n0 + n]
                )
        hts = [hpool.tile([PART, C], _F32R, tag="h", name=f"ht{i}") for i in range(MF)]

        # Phase 1: hT[mf] = silu(w1.T @ xT) * (w3.T @ xT), weight-stationary
        for mf in range(MF):
            if mf == 0:
                w1t, w3t = w1t0, w3t0
            else:
                w1t = w1pool.tile([PART, KD * PART], _F32R, tag="w1")
                nc.sync.dma_start(w1t[:], w1_d.ap()[mf])
                w3t = w3pool.tile([PART, KD * PART], _F32R, tag="w3")
                nc.sync.dma_start(w3t[:], w3_d.ap()[mf])
            for n0, n in chunks:
                p1 = p1pool.tile([PART, 512], _F32, tag="p1")
                p3 = p3pool.tile([PART, 512], _F32, tag="p3")
                for kd in range(KD):
                    nc.tensor.matmul(
                        p1[:, :n],
                        w1t[:, bass.ts(kd, PART)],
                        xts[kd][:, n0 : n0 + n],
                        start=(kd == 0),
                        stop=(kd == KD - 1),
                    )
                for kd in range(KD):
                    nc.tensor.matmul(
                        p3[:, :n],
                        w3t[:, bass.ts(kd, PART)],
                        xts[kd][:, n0 : n0 + n],
                        start=(kd == 0),
                        stop=(kd == KD - 1),
                    )
                tmp = tmppool.tile([PART, 512], _F32R, tag="tmp")
                nc.scalar.activation(
                    tmp[:, :n], p1[:, :n], mybir.ActivationFunctionType.Silu
                )
                nc.vector.tensor_mul(hts[mf][:, n0 : n0 + n], tmp[:, :n], p3[:, :n])

        # Phase 2: yT[md] = w2.T @ hT
        for md in range(MD):
            w2t = w2pool.tile([PART, KF * PART], _F32R, tag="w2")
            nc.sync.dma_start(w2t[:], w2_d.ap()[md])
            yt = ypool.tile([PART, C], _F32, tag="y")
            for n0, n in chunks:
                p2 = p2pool.tile([PART, 512], _F32, tag="p2")
                for kf in range(KF):
                    nc.tensor.matmul(
                        p2[:, :n],
                        w2t[:, bass.ts(kf, PART)],
                        hts[kf][:, n0 : n0 + n],
                        start=(kf == 0),
                        stop=(kf == KF - 1),
                    )
                nc.vector.tensor_copy(yt[:, n0 : n0 + n], p2[:, :n])
                nc.sync.dma_start(
                    yt_d.ap()[md][:, n0 : n0 + n], yt[:, n0 : n0 + n]
                )

    nc.compile()
    return nc


def _get_nc(C):
    if C not in _nc_cache:
        _nc_cache[C] = _build(C)
    return _nc_cache[C]


def _route(x, gate_w):
    """Gate + top-2 routing, matching reference numerics (jax on CPU)."""
    import jax
    import jax.numpy as jnp

    cpu = jax.devices("cpu")[0]
    with jax.default_device(cpu):
        xj = jnp.asarray(x)
        gj = jnp.asarray(gate_w)
        probs = jax.nn.softmax(xj @ gj, axis=-1)
        top_w, top_i = jax.lax.top_k(probs, TOP_K)
        top_w = top_w / jnp.sum(top_w, axis=-1, keepdims=True)
        return np.asarray(top_w), np.asarray(top_i)


def _pack_w(w, m_tiles, k_tiles):
    """[K, M] -> [m_tiles, 128, k_tiles*128]: per m-tile, SBUF layout
    [k-partition(128), (k-tile, m-col)]."""
    return np.ascontiguousarray(
        w.reshape(k_tiles, PART, m_tiles, PART)
        .transpose(2, 1, 0, 3)
        .reshape(m_tiles, PART, k_tiles * PART)
    )


def kernel(hidden_states, gate_w, w1, w2, w3):
    hidden_states = np.asarray(hidden_states, dtype=np.float32)
    gate_w = np.asarray(gate_w, dtype=np.float32)
    w1 = np.asarray(w1, dtype=np.float32)
    w2 = np.asarray(w2, dtype=np.float32)
    w3 = np.asarray(w3, dtype=np.float32)

    orig_shape = hidden_states.shape
    x = hidden_states.reshape(-1, D)  # [T, D]
    T = x.shape[0]

    top_w, top_i = _route(x, gate_w)

    # Dispatch: group (token, slot) pairs by expert, token order preserved
    flat_e = top_i.reshape(-1)
    flat_w = top_w.reshape(-1)
    tok_of = np.repeat(np.arange(T), TOP_K)
    order = np.argsort(flat_e, kind="stable")
    sorted_tok = tok_of[order]
    sorted_w = flat_w[order]
    counts = np.bincount(flat_e, minlength=E)
    starts = np.concatenate([[0], np.cumsum(counts)])
    # Capacity: even (f32r ISA) and capped at 1024 so the kernel runs two
    # full 512-wide psum chunks (a third chunk sweep would re-pay all weight
    # loads). The few tokens above capacity are combined on the host below.
    C = min(1024, max(256, int(counts.max() + 1) // 2 * 2))

    in_maps = []
    for e in range(E):
        idx = sorted_tok[starts[e] : starts[e + 1]][:C]
        xt = np.zeros((D, C), np.float32)
        xt[:, : len(idx)] = x[idx].T
        in_maps.append(
            {
                "xt": np.ascontiguousarray(xt.reshape(KD, PART, C)),
                "w1t": _pack_w(w1[e], MF, KD),
                "w3t": _pack_w(w3[e], MF, KD),
                "w2t": _pack_w(w2[e], MD, KF),
            }
        )

    nc = _get_nc(C)
    res = run_bass_kernel_spmd(
        nc, in_maps, core_ids=list(range(N_CORES)), trace=TRACE
    )
    kernel.last_result = res

    def _silu(v):
        return v / (1.0 + np.exp(-v))

    out = np.zeros((T, D), np.float32)
    for e in range(E):
        idx = sorted_tok[starts[e] : starts[e + 1]]
        wts = sorted_w[starts[e] : starts[e + 1]]
        yt = res.results[e]["yt"].reshape(D, C)
        n_dev = min(len(idx), C)
        out[idx[:n_dev]] += yt[:, :n_dev].T * wts[:n_dev, None]
        if len(idx) > C:  # capacity overflow: exact host FFN for the tail
            xo = x[idx[C:]]
            yo = (_silu(xo @ w1[e]) * (xo @ w3[e])) @ w2[e]
            out[idx[C:]] += yo * wts[C:, None]
    return out.reshape(orig_shape)
